# revision 1
# baseline (speedup 1.0000x reference)
"""Trainium2 Bass kernel for nn_MultiHeadCrossAttention_47519518163418.

Sharding: 8 cores = (batch b in {0,1}) x (head h in {0..3}); core c: b=c//4, h=c%4.
Each core computes q/k/v for its head's 32 channels (conv output channels are
independent), runs the full 4096x4096 attention for that head (flash-style,
scores computed transposed so no transposes of the score matrix are needed,
softmax without max-subtraction, row sums via an appended ones-column in the
PV matmul), then the cores of a batch AllGather the attention output to form
the full 128-channel mha2d. Green (upsample+conv+ILN+silu) and purple
(upsample+conv+ILN+sigmoid, gated by s) paths are computed per-core for the
core's 32 output channels using a phase-collapsed 2x2-tap decomposition of
"upsample2 + reflect-pad + 3x3 conv" (which reduces to edge-clamp padding on
the original-resolution image). ILN layer statistics are combined with one
tiny AllReduce. Host precomputes positional encodings + paddings and
reassembles the sharded outputs.
"""

import sys

if "/opt/trn_rl_repo" not in sys.path:
    sys.path.insert(0, "/opt/trn_rl_repo")

import numpy as np

NUM_HEADS = 4
EPS = 1e-5
D_HEAD = 32
SCALE = float(D_HEAD) ** -0.5
N_PX = 16384.0          # pixels per channel of the upsampled image
N_TOT = 128 * 16384.0   # elements per batch for layer stats

_CORES = list(range(8))
_REPLICA_GROUPS = [[0, 1, 2, 3], [4, 5, 6, 7]]


# ----------------------------------------------------------------------------
# Host-side helpers
# ----------------------------------------------------------------------------

def pos_encoding_pe(c, L, dtype=np.float32):
    half = c // 2
    pos = np.arange(L, dtype=dtype)
    depths = np.arange(half, dtype=dtype) / half
    rates = 1.0 / (10000.0 ** depths)
    ang = pos[:, None] * rates[None, :]
    pe = np.concatenate([np.sin(ang), np.cos(ang)], axis=-1)  # [L, c]
    return pe.T.astype(dtype)  # [c, L]


def reflect_pad(x):
    return np.pad(x, ((0, 0), (1, 1), (1, 1)), mode="reflect")


def edge_pad(x):
    return np.pad(x, ((0, 0), (1, 1), (1, 1)), mode="edge")


_KSET = {(0, 0): [0], (0, 1): [1, 2], (1, 0): [0, 1], (1, 1): [2]}


def collapse_w2(w):
    """w [co, ci, 3, 3] -> W2 [4 (p=2*pr+pc), 2 (dy), 2 (dx), ci, co]."""
    co, ci = w.shape[0], w.shape[1]
    W2 = np.zeros((4, 2, 2, ci, co), dtype=w.dtype)
    for pr in range(2):
        for pc in range(2):
            p = 2 * pr + pc
            for dy in range(2):
                for dx in range(2):
                    acc = np.zeros((co, ci), dtype=np.float64)
                    for ky in _KSET[(pr, dy)]:
                        for kx in _KSET[(pc, dx)]:
                            acc = acc + w[:, :, ky, kx].astype(np.float64)
                    W2[p, dy, dx] = acc.T.astype(w.dtype)
    return W2


def arrange_to_strips(x2d):
    """x [32, 128, 128] -> arranged [128, 4096] phase-major: partition
    32*(2*pr+pc)+c, free r*64+cc for upsampled pixel (2r+pr, 2cc+pc)."""
    t = x2d.reshape(32, 64, 2, 64, 2)          # c, r, pr, cc, pc
    t = t.transpose(2, 4, 0, 1, 3)              # pr, pc, c, r, cc
    return np.ascontiguousarray(t.reshape(128, 4096))


def unarrange_from_strips(arr):
    t = arr.reshape(2, 2, 32, 64, 64)           # pr, pc, c, r, cc
    t = t.transpose(2, 3, 0, 4, 1)              # c, r, pr, cc, pc
    return np.ascontiguousarray(t.reshape(32, 128, 128))


_PE_Y = None
_PE_S = None


_BATCH_CACHE = {}


def _batch_shared(inputs, b):
    """Padded/PE-added tensors shared by the 4 cores of a batch."""
    key = (id(inputs), b)
    if key in _BATCH_CACHE:
        return _BATCH_CACHE[key]
    y = np.asarray(inputs["y"], dtype=np.float32)[b]
    s = np.asarray(inputs["s"], dtype=np.float32)[b]
    ypepad = np.ascontiguousarray(
        reflect_pad((y + _PE_Y).astype(np.float32)).reshape(2, 128, 66, 66))
    yreppad = np.ascontiguousarray(edge_pad(y).reshape(2, 128, 66, 66))
    spepad = np.ascontiguousarray(reflect_pad((s + _PE_S).astype(np.float32)))
    _BATCH_CACHE.clear()
    _BATCH_CACHE[key] = (ypepad, yreppad, spepad)
    return _BATCH_CACHE[key]


def prepare_core_inputs(inputs, core):
    global _PE_Y, _PE_S
    if _PE_Y is None:
        _PE_Y = pos_encoding_pe(256, 64 * 64).reshape(256, 64, 64)
        _PE_S = pos_encoding_pe(128, 128 * 128).reshape(128, 128, 128)
    b, h = core // 4, core % 4
    ch = slice(32 * h, 32 * h + 32)
    s = np.asarray(inputs["s"], dtype=np.float32)[b]

    ypepad, yreppad, spepad = _batch_shared(inputs, b)
    sgate = arrange_to_strips(np.ascontiguousarray(s[ch]))

    w_blue_y = np.asarray(inputs["w_blue_y"], dtype=np.float32)[ch]
    w_blue_s = np.asarray(inputs["w_blue_s"], dtype=np.float32)[ch]
    w_green = np.asarray(inputs["w_green"], dtype=np.float32)[ch]
    w_purple = np.asarray(inputs["w_purple"], dtype=np.float32)[ch]

    wq = np.zeros((18, 128, 128), dtype=np.float32)
    for t in range(9):
        ky, kx = t // 3, t % 3
        for kt in range(2):
            blk = w_blue_y[:, 128 * kt : 128 * kt + 128, ky, kx].T
            wq[t * 2 + kt] = np.tile(blk, (1, 4))
    wv = np.zeros((9, 128, 32), dtype=np.float32)
    for t in range(9):
        ky, kx = t // 3, t % 3
        wv[t] = w_blue_s[:, :, ky, kx].T

    def make_w9(w):
        # W9[ey, ex][ci, 32*p+c] = W2[p, ey-pr, ex-pc][ci, c] (0 if invalid):
        # all four phases computed from one 9-tap pass over the edge-padded
        # original-resolution image, phase-major on output partitions.
        W2 = collapse_w2(w)                      # [4, 2, 2, ci, co32]
        ci = W2.shape[3]
        W9 = np.zeros((3, 3, ci, 128), dtype=np.float32)
        for p in range(4):
            pr, pc = p // 2, p % 2
            for dy in range(2):
                for dx in range(2):
                    W9[pr + dy, pc + dx, :, 32 * p : 32 * p + 32] = W2[p, dy, dx]
        return W9

    W9g = make_w9(w_green)                       # [3, 3, 256, 128]
    wg = W9g.reshape(3, 3, 2, 128, 128).transpose(0, 1, 2, 3, 4).reshape(18, 128, 128).copy()
    W9p = make_w9(w_purple)                      # [3, 3, 128, 128]
    wp = W9p.reshape(9, 128, 128).copy()

    affg = np.stack(
        [np.asarray(inputs["rho_g"], dtype=np.float32).reshape(128)[ch],
         np.asarray(inputs["gamma_g"], dtype=np.float32).reshape(128)[ch],
         np.asarray(inputs["beta_g"], dtype=np.float32).reshape(128)[ch]],
        axis=1)
    affp = np.stack(
        [np.asarray(inputs["rho_p"], dtype=np.float32).reshape(128)[ch],
         np.asarray(inputs["gamma_p"], dtype=np.float32).reshape(128)[ch],
         np.asarray(inputs["beta_p"], dtype=np.float32).reshape(128)[ch]],
        axis=1)

    sel = np.zeros((128, 32), dtype=np.float32)
    for p in range(128):
        sel[p, p % 32] = 1.0

    return {
        "ypepad": ypepad,
        "yreppad": yreppad,
        "spepad": spepad,
        "sgate": sgate,
        "wq": wq,
        "wv": wv,
        "wg": wg,
        "wp": wp,
        "affg": np.ascontiguousarray(affg),
        "affp": np.ascontiguousarray(affp),
        "sel": sel,
        "onesr": np.ones((128, 1), dtype=np.float32),
    }


def assemble_output(per_core_z, per_core_upy):
    out = np.zeros((2, 256, 128, 128), dtype=np.float32)
    for core in range(8):
        b, h = core // 4, core % 4
        out[b, 32 * h : 32 * h + 32] = unarrange_from_strips(per_core_z[core])
        out[b, 128 + 32 * h : 128 + 32 * h + 32] = unarrange_from_strips(per_core_upy[core])
    return out


# ----------------------------------------------------------------------------
# Bass kernel
# ----------------------------------------------------------------------------

def build_bass(loop_n=None, no_cc=False):
    import concourse.bass as bass
    import concourse.tile as tile
    from concourse import bacc, mybir

    f32 = mybir.dt.float32
    f32r = mybir.dt.float32r
    AF = mybir.ActivationFunctionType
    ALU = mybir.AluOpType

    def r32(ap):
        return ap.bitcast(f32r)

    nc = bacc.Bacc(num_devices=8)

    # ---- I/O ----
    ypepad_d = nc.declare_dram_parameter("ypepad", [2, 128, 66, 66], f32r, isOutput=False)
    yreppad_d = nc.declare_dram_parameter("yreppad", [2, 128, 66, 66], f32r, isOutput=False)
    spepad_d = nc.declare_dram_parameter("spepad", [128, 130, 130], f32r, isOutput=False)
    sgate_d = nc.declare_dram_parameter("sgate", [128, 4096], f32, isOutput=False)
    wq_d = nc.declare_dram_parameter("wq", [18, 128, 128], f32r, isOutput=False)
    wv_d = nc.declare_dram_parameter("wv", [9, 128, 32], f32r, isOutput=False)
    wg_d = nc.declare_dram_parameter("wg", [18, 128, 128], f32r, isOutput=False)
    wp_d = nc.declare_dram_parameter("wp", [9, 128, 128], f32r, isOutput=False)
    affg_d = nc.declare_dram_parameter("affg", [32, 3], f32, isOutput=False)
    affp_d = nc.declare_dram_parameter("affp", [32, 3], f32, isOutput=False)
    sel_d = nc.declare_dram_parameter("sel", [128, 32], f32, isOutput=False)
    onesr_d = nc.declare_dram_parameter("onesr", [128, 1], f32r, isOutput=False)
    zout_d = nc.declare_dram_parameter("zout", [128, 4096], f32, isOutput=True)
    upyout_d = nc.declare_dram_parameter("upyout", [128, 4096], f32, isOutput=True)

    # ---- internal DRAM (collective bounce buffers etc.) ----
    cc1_in = nc.dram_tensor("cc1_in", [32, 4096], f32r)
    cc1_out = nc.dram_tensor("cc1_out", [128, 4096], f32r)
    cc2_in = nc.dram_tensor("cc2_in", [1, 4], f32)
    cc2_out = nc.dram_tensor("cc2_out", [1, 4], f32)
    rsumb = nc.dram_tensor("rsumb", [4096], f32)
    rsumb2 = nc.dram_tensor("rsumb2", [4096], f32)

    import contextlib

    with tile.TileContext(nc) as tc, contextlib.ExitStack() as ctx:
        pers = ctx.enter_context(tc.tile_pool(name="pers", bufs=1))
        small = ctx.enter_context(tc.tile_pool(name="small", bufs=1))

        # ---------------- weights + constants ----------------
        wq_sb = pers.tile([128, 18, 128], f32r, tag="wq")
        nc.sync.dma_start(out=wq_sb, in_=wq_d[:, :, :].rearrange("t p m -> p t m"))
        wv_sb = pers.tile([128, 9, 32], f32r, tag="wv")
        nc.sync.dma_start(out=wv_sb, in_=wv_d[:, :, :].rearrange("t p m -> p t m"))
        wg_sb = pers.tile([128, 18, 128], f32r, tag="wg")
        nc.sync.dma_start(out=wg_sb, in_=wg_d[:, :, :].rearrange("t p m -> p t m"))
        wp_sb = pers.tile([128, 9, 128], f32r, tag="wp")
        nc.sync.dma_start(out=wp_sb, in_=wp_d[:, :, :].rearrange("t p m -> p t m"))
        sel_sb = pers.tile([128, 32], f32, tag="sel")
        nc.sync.dma_start(out=sel_sb, in_=sel_d[:, :])
        ones_sb = pers.tile([128, 1], f32, tag="ones")
        nc.vector.memset(ones_sb, 1.0)
        affg_sb = small.tile([32, 3], f32, tag="affg")
        nc.sync.dma_start(out=affg_sb, in_=affg_d[:, :])
        affp_sb = small.tile([32, 3], f32, tag="affp")
        nc.sync.dma_start(out=affp_sb, in_=affp_d[:, :])

        def rsqrt_col(x, p, tag, eps=EPS):
            """[p, 1] tile -> rsqrt(x + eps), via reciprocal + Sqrt ACT +
            one Newton step (y*(1.5 - 0.5*xe*y^2)) to clean up sqrt ULPs."""
            xe = small.tile([p, 1], f32, tag=tag + "xe", name=tag + "xe")
            nc.vector.tensor_scalar_add(xe, x, eps)
            r = small.tile([p, 1], f32, tag=tag + "r", name=tag + "r")
            nc.vector.reciprocal(out=r, in_=xe)
            y = small.tile([p, 1], f32, tag=tag + "y", name=tag + "y")
            nc.scalar.activation(out=y, in_=r, func=AF.Sqrt)
            t = small.tile([p, 1], f32, tag=tag + "nt", name=tag + "nt")
            nc.vector.tensor_mul(t, y, y)
            nc.vector.tensor_mul(t, t, xe)
            nc.vector.tensor_scalar(out=t, in0=t, scalar1=-0.5, scalar2=1.5,
                                    op0=ALU.mult, op1=ALU.add)
            nc.vector.tensor_mul(y, y, t)
            return y

        def emit_body():
            attn = ctx.enter_context(tc.tile_pool(name="attn", bufs=1))
            # =========== v conv (blue_s, stride 2, M=32) ===========
            vT_sb = attn.tile([128, 33 * 32], f32r, tag="vT")
            vT_ones_view = vT_sb.rearrange("p (jb c) -> p jb c", c=33)[:, :, 32:33]
            nc.sync.dma_start(
                out=vT_ones_view,
                in_=bass.AP(tensor=onesr_d, offset=0, ap=[[1, 128], [0, 32], [0, 1]]))
            vstats = small.tile([32, 8, 6], f32, tag="vstats")
            with tc.tile_pool(name="vsec", bufs=1) as vsec, \
                 tc.tile_pool(name="vtmp", bufs=2) as vtmp_pool, \
                 tc.tile_pool(name="cps2", bufs=3, space="PSUM") as cps2:
                spe = vsec.tile([128, 130, 130], f32r, tag="spe")
                for rb in range(5):
                    r0, r1 = 26 * rb, 26 * rb + 26
                    nc.sync.dma_start(out=spe[:, r0:r1, :], in_=spepad_d[:, r0:r1, :])
                vraw = vtmp_pool.tile([32, 4096], f32, tag="vtmp", name="vraw")
                for chunk in range(8):
                    vps = cps2.tile([128, 512], f32, tag="vps")
                    r0 = 8 * chunk
                    for t in range(9):
                        ky, kx = t // 3, t % 3
                        nc.tensor.matmul(
                            vps[0:32, :],
                            wv_sb[:, t, :],
                            spe[:, 2 * r0 + ky : 2 * r0 + ky + 16 : 2, kx : kx + 128 : 2],
                            start=(t == 0), stop=(t == 8),
                        )
                    nc.vector.tensor_copy(vraw[:, 512 * chunk : 512 * chunk + 512], vps[0:32, :])
                    nc.vector.bn_stats(out=vstats[:, chunk, :], in_=vraw[:, 512 * chunk : 512 * chunk + 512])

                vmv = small.tile([32, 2], f32, tag="vmv")
                nc.vector.bn_aggr(out=vmv, in_=vstats)
                vinv = rsqrt_col(vmv[:, 1:2], 32, "vinv")
                vbias = small.tile([32, 1], f32, tag="vbias")
                nc.vector.tensor_scalar(out=vbias, in0=vmv[:, 0:1], scalar1=vinv, scalar2=-1.0,
                                        op0=ALU.mult, op1=ALU.mult)
                v2d = vtmp_pool.tile([32, 4096], f32, tag="vtmp", name="v2d")
                nc.scalar.activation(out=v2d, in_=vraw, func=AF.Silu, bias=vbias, scale=vinv)

                # vT with ones column: vT_sb[32w+i, 33*jb+c] = v2d[c, 128*jb+32w+i]
                vt32 = vtmp_pool.tile([32, 4096], f32, tag="vtmp", name="vt32")
                nc.vector.transpose(out=vt32, in_=v2d)
                vt32_v = vt32.rearrange("p (m c) -> p m c", c=32)   # m = 4*jb + w
                vT_v = vT_sb.rearrange("p (jb c) -> p jb c", c=33)
                for w in range(4):
                    nc.gpsimd.dma_start(
                        out=vT_v[32 * w : 32 * w + 32, :, 0:32],
                        in_=vt32_v[:, w::4, :])

            # =========== q conv (blue_y, M=128 replicated) ===========
            qstats = small.tile([128, 8, 6], f32, tag="qstats")
            with tc.tile_pool(name="qsec", bufs=1) as qsec, \
                 tc.tile_pool(name="cps1", bufs=3, space="PSUM") as cps1:
                ype = [qsec.tile([128, 66, 66], f32r, tag=f"ype{kt}", name=f"ype{kt}") for kt in range(2)]
                for kt in range(2):
                    for rb in range(3):
                        r0, r1 = 22 * rb, 22 * rb + 22
                        nc.sync.dma_start(out=ype[kt][:, r0:r1, :], in_=ypepad_d[kt][:, r0:r1, :])
                qraw = qsec.tile([128, 4096], f32, tag="qraw")
                for chunk in range(8):
                    qps = cps1.tile([128, 512], f32, tag="qps")
                    r0 = 8 * chunk
                    idx = 0
                    for t in range(9):
                        ky, kx = t // 3, t % 3
                        for kt in range(2):
                            nc.tensor.matmul(
                                qps[:, :],
                                wq_sb[:, t * 2 + kt, :],
                                ype[kt][:, r0 + ky : r0 + ky + 8, kx : kx + 64],
                                start=(idx == 0), stop=(idx == 17),
                            )
                            idx += 1
                    nc.vector.tensor_copy(qraw[:, 512 * chunk : 512 * chunk + 512], qps[:, :])
                    nc.vector.bn_stats(out=qstats[:, chunk, :], in_=qraw[:, 512 * chunk : 512 * chunk + 512])

                qmv = small.tile([128, 2], f32, tag="qmv")
                nc.vector.bn_aggr(out=qmv, in_=qstats)
                qinv = rsqrt_col(qmv[:, 1:2], 128, "qinv")
                qbias = small.tile([128, 1], f32, tag="qbias")
                nc.vector.tensor_scalar(out=qbias, in0=qmv[:, 0:1], scalar1=qinv, scalar2=-1.0,
                                        op0=ALU.mult, op1=ALU.mult)
                qrep = attn.tile([128, 4096], f32r, tag="qrep")
                nc.scalar.activation(out=qrep, in_=qraw, func=AF.Silu, bias=qbias, scale=qinv)

            # =========== attention + interleaved green conv ===========
            greenraw = ctx.enter_context(tc.tile_pool(name="gpool", bufs=1)).tile(
                [128, 4096], f32, tag="greenraw", name="greenraw")
            gstats = small.tile([128, 8, 6], f32, tag="gstats")
            mharaw = attn.tile([33, 4096], f32, tag="mharaw")

            with tc.tile_pool(name="yrep", bufs=1) as yrep_pool, \
                 tc.tile_pool(name="aexpp", bufs=4) as aexp_pool, \
                 tc.tile_pool(name="gps", bufs=2, space="PSUM") as gps_pool, \
                 tc.tile_pool(name="qkps", bufs=2, space="PSUM") as qkps, \
                 tc.tile_pool(name="pvps", bufs=2, space="PSUM") as pvps:
                yrep = [yrep_pool.tile([128, 66, 66], f32r, tag=f"yrep{kt}", name=f"yrep{kt}") for kt in range(2)]
                for kt in range(2):
                    for rb in range(3):
                        r0, r1 = 22 * rb, 22 * rb + 22
                        nc.sync.dma_start(out=yrep[kt][:, r0:r1, :], in_=yreppad_d[kt][:, r0:r1, :])

                green_tiles = {}

                def green_piece(piece):
                    # piece = (chunk, sub) with sub in 0..5 -> 3 MMs each
                    chunk, sub = piece // 6, piece % 6
                    r0 = 8 * chunk
                    if sub == 0:
                        green_tiles[chunk] = gps_pool.tile(
                            [128, 512], f32, tag="gpsum", name=f"g{chunk}")
                    gtile = green_tiles[chunk]
                    for k in range(3):
                        idx = sub * 3 + k
                        tap, kt = idx // 2, idx % 2
                        ey, ex = tap // 3, tap % 3
                        nc.tensor.matmul(
                            gtile[:, :],
                            wg_sb[:, tap * 2 + kt, :],
                            yrep[kt][:, r0 + ey : r0 + ey + 8, ex : ex + 64],
                            start=(idx == 0), stop=(idx == 17),
                        )
                    if sub == 5:
                        col = 512 * chunk
                        nc.vector.tensor_copy(greenraw[:, col : col + 512], gtile[:, :])
                        nc.vector.bn_stats(out=gstats[:, chunk, :], in_=greenraw[:, col : col + 512])
                        del green_tiles[chunk]

                vT_v = vT_sb.rearrange("p (jb c) -> p jb c", c=33)
                gu_next = 0
                it = 0
                for I in range(8):
                    pvt = pvps.tile([128, 512], f32, tag="pvt", name=f"pvt{I}")
                    for g in range(16):
                        qk = qkps.tile([128, 1024], f32, tag="qk", name=f"qk{I}_{g}")
                        for t in range(2):
                            jb = 2 * g + t
                            nc.tensor.matmul(
                                qk[:, 512 * t : 512 * t + 512],
                                qrep[0:32, 128 * jb : 128 * jb + 128],
                                qrep[0:32, 512 * I : 512 * I + 512],
                                start=True, stop=True,
                            )
                        aexp = aexp_pool.tile([128, 1024], f32r, tag="aexp", name=f"ae{I}_{g}")
                        nc.scalar.activation(out=aexp, in_=qk, func=AF.Exp, scale=SCALE)
                        for t in range(2):
                            jb = 2 * g + t
                            nc.tensor.matmul(
                                pvt[0:33, :],
                                vT_v[:, jb, :],
                                aexp[:, 512 * t : 512 * t + 512],
                                start=(g == 0 and t == 0), stop=(g == 15 and t == 1),
                                skip_group_check=True,
                            )
                        it += 1
                        if it % 2 == 0 and gu_next < 48:
                            green_piece(gu_next)
                            gu_next += 1
                    nc.vector.tensor_copy(mharaw[:, 512 * I : 512 * I + 512], pvt[0:33, :])

            gmv = small.tile([128, 2], f32, tag="gmv")
            nc.vector.bn_aggr(out=gmv, in_=gstats)

            # =========== softmax denominators + divide ===========
            with tc.tile_pool(name="divp", bufs=1) as divp:
                nc.sync.dma_start(out=bass.AP(tensor=rsumb, offset=0, ap=[[1, 4096]]),
                                  in_=mharaw[32:33, :])
                rsq = small.tile([128, 32], f32, tag="rsq")
                nc.sync.dma_start(out=rsq, in_=bass.AP(tensor=rsumb, offset=0, ap=[[32, 128], [1, 32]]))
                nc.vector.reciprocal(out=rsq, in_=rsq)
                nc.sync.dma_start(out=bass.AP(tensor=rsumb2, offset=0, ap=[[32, 128], [1, 32]]), in_=rsq)
                rs32 = divp.tile([32, 4096], f32, tag="rs32")
                nc.sync.dma_start(out=rs32,
                                  in_=bass.AP(tensor=rsumb2, offset=0, ap=[[0, 32], [1, 4096]]))
                mha2db = divp.tile([32, 4096], f32r, tag="mha2db")
                nc.vector.tensor_mul(mha2db, mharaw[0:32, :], rs32)

                # AllGather mha across the 4 cores of this batch
                nc.sync.dma_start(out=cc1_in[:, :], in_=mha2db)
                if no_cc:
                    for g in range(4):
                        nc.sync.dma_start(out=cc1_out[32 * g : 32 * g + 32, :], in_=cc1_in[:, :])
                else:
                    nc.gpsimd.collective_compute(
                        "AllGather", mybir.AluOpType.bypass,
                        replica_groups=_REPLICA_GROUPS,
                        ins=[cc1_in[:, :]],
                        outs=[cc1_out[:, :]],
                    )

            # =========== purple conv ===========
            with tc.tile_pool(name="tailp", bufs=1) as tailp, \
                 tc.tile_pool(name="gps2", bufs=2, space="PSUM") as gps2, \
                 tc.tile_pool(name="tailps", bufs=2, space="PSUM") as tailps:
                mhapad = tailp.tile([128, 66, 66], f32r, tag="mhapad")
                cc1_v = cc1_out.rearrange("p (r c) -> p r c", c=64)
                nc.sync.dma_start(out=mhapad[:, 1:65, 1:65], in_=cc1_v)
                nc.sync.dma_start(out=mhapad[:, 0:1, 1:65], in_=cc1_v[:, 0:1, :])
                nc.sync.dma_start(out=mhapad[:, 65:66, 1:65], in_=cc1_v[:, 63:64, :])
                nc.sync.dma_start(out=mhapad[:, 0:66, 0:1], in_=mhapad[:, 0:66, 1:2])
                nc.sync.dma_start(out=mhapad[:, 0:66, 65:66], in_=mhapad[:, 0:66, 64:65])

                purpleraw = tailp.tile([128, 4096], f32, tag="purpleraw")
                pstats = small.tile([128, 8, 6], f32, tag="pstats")
                for chunk in range(8):
                    ptile = gps2.tile([128, 512], f32, tag="gpsum2", name=f"pt{chunk}")
                    r0 = 8 * chunk
                    for tap in range(9):
                        ey, ex = tap // 3, tap % 3
                        nc.tensor.matmul(
                            ptile[:, :],
                            wp_sb[:, tap, :],
                            mhapad[:, r0 + ey : r0 + ey + 8, ex : ex + 64],
                            start=(tap == 0), stop=(tap == 8),
                        )
                    col = 512 * chunk
                    nc.vector.tensor_copy(purpleraw[:, col : col + 512], ptile[:, :])
                    nc.vector.bn_stats(out=pstats[:, chunk, :], in_=purpleraw[:, col : col + 512])
                pmv = small.tile([128, 2], f32, tag="pmv")
                nc.vector.bn_aggr(out=pmv, in_=pstats)

                # ---- stats -> sums, channel combine, allreduce ----
                def part_sums(mv, tag):
                    s2 = small.tile([128, 2], f32, tag=tag, name=tag)
                    nc.vector.tensor_scalar_mul(s2[:, 0:1], mv[:, 0:1], 4096.0)
                    t = small.tile([128, 1], f32, tag=tag + "t", name=tag + "t")
                    nc.vector.tensor_mul(t, mv[:, 0:1], mv[:, 0:1])
                    nc.vector.tensor_add(t, t, mv[:, 1:2])
                    nc.vector.tensor_scalar_mul(s2[:, 1:2], t, 4096.0)
                    return s2

                gsums2 = part_sums(gmv, "gsums2")
                psums2 = part_sums(pmv, "psums2")

                chps = tailps.tile([128, 512], f32, tag="tps", name="chps")
                nc.tensor.matmul(chps[0:32, 0:2], sel_sb, gsums2, start=True, stop=True)
                gch = small.tile([32, 2], f32, tag="gch")
                nc.vector.tensor_copy(gch, chps[0:32, 0:2])
                chps2 = tailps.tile([128, 512], f32, tag="tps", name="chps2")
                nc.tensor.matmul(chps2[0:32, 0:2], sel_sb, psums2, start=True, stop=True)
                pch = small.tile([32, 2], f32, tag="pch")
                nc.vector.tensor_copy(pch, chps2[0:32, 0:2])

                lps = tailps.tile([128, 512], f32, tag="tps", name="lps")
                nc.tensor.matmul(lps[0:1, 0:2], ones_sb, gsums2, start=True, stop=True)
                nc.tensor.matmul(lps[0:1, 2:4], ones_sb, psums2, start=True, stop=True)
                lsb = small.tile([1, 4], f32, tag="lsb")
                nc.vector.tensor_copy(lsb, lps[0:1, 0:4])
                nc.sync.dma_start(out=cc2_in[:, :], in_=lsb)
                if no_cc:
                    nc.sync.dma_start(out=cc2_out[:, :], in_=cc2_in[:, :])
                else:
                    nc.gpsimd.collective_compute(
                        "AllReduce", mybir.AluOpType.add,
                        replica_groups=_REPLICA_GROUPS,
                        ins=[cc2_in[:, :]],
                        outs=[cc2_out[:, :]],
                    )
                lng = small.tile([32, 4], f32, tag="lng")
                nc.sync.dma_start(out=lng, in_=bass.AP(tensor=cc2_out, offset=0, ap=[[0, 32], [1, 4]]))

                # ---- ILN affines ----
                def iln_affine(ch_sums, S_col, aff_sb, tag):
                    n, n1 = N_PX, N_PX - 1.0
                    nt, nt1 = N_TOT, N_TOT - 1.0
                    in_m = small.tile([32, 1], f32, tag=tag + "im", name=tag + "im")
                    nc.vector.tensor_scalar_mul(in_m, ch_sums[:, 0:1], 1.0 / n)
                    t1 = small.tile([32, 1], f32, tag=tag + "t1", name=tag + "t1")
                    nc.vector.tensor_mul(t1, ch_sums[:, 0:1], ch_sums[:, 0:1])
                    nc.vector.tensor_scalar_mul(t1, t1, 1.0 / n)
                    nc.vector.tensor_sub(t1, ch_sums[:, 1:2], t1)
                    in_v = small.tile([32, 1], f32, tag=tag + "iv", name=tag + "iv")
                    nc.vector.tensor_scalar_mul(in_v, t1, 1.0 / n1)
                    inv_in = rsqrt_col(in_v, 32, tag + "ii")

                    ln_m = small.tile([32, 1], f32, tag=tag + "lm", name=tag + "lm")
                    nc.vector.tensor_scalar_mul(ln_m, S_col[:, 0:1], 1.0 / nt)
                    l1 = small.tile([32, 1], f32, tag=tag + "l1", name=tag + "l1")
                    nc.vector.tensor_mul(l1, S_col[:, 0:1], S_col[:, 0:1])
                    nc.vector.tensor_scalar_mul(l1, l1, 1.0 / nt)
                    nc.vector.tensor_sub(l1, S_col[:, 1:2], l1)
                    ln_v = small.tile([32, 1], f32, tag=tag + "lv", name=tag + "lv")
                    nc.vector.tensor_scalar_mul(ln_v, l1, 1.0 / nt1)
                    inv_ln = rsqrt_col(ln_v, 32, tag + "il")

                    rho = aff_sb[:, 0:1]
                    t3 = small.tile([32, 1], f32, tag=tag + "t3", name=tag + "t3")
                    nc.vector.tensor_mul(t3, rho, inv_in)
                    t6 = small.tile([32, 1], f32, tag=tag + "t6", name=tag + "t6")
                    nc.vector.tensor_mul(t6, rho, inv_ln)
                    nc.vector.tensor_sub(t6, inv_ln, t6)
                    A = small.tile([32, 1], f32, tag=tag + "A", name=tag + "A")
                    nc.vector.tensor_add(A, t3, t6)
                    u1 = small.tile([32, 1], f32, tag=tag + "u1", name=tag + "u1")
                    nc.vector.tensor_mul(u1, in_m, t3)
                    u2 = small.tile([32, 1], f32, tag=tag + "u2", name=tag + "u2")
                    nc.vector.tensor_mul(u2, ln_m, t6)
                    nc.vector.tensor_add(u1, u1, u2)
                    B = small.tile([32, 1], f32, tag=tag + "B", name=tag + "B")
                    nc.vector.tensor_scalar_mul(B, u1, -1.0)
                    sb = small.tile([32, 2], f32, tag=tag + "sb", name=tag + "sb")
                    nc.vector.tensor_mul(sb[:, 0:1], A, aff_sb[:, 1:2])
                    nc.vector.tensor_mul(sb[:, 1:2], B, aff_sb[:, 1:2])
                    nc.vector.tensor_add(sb[:, 1:2], sb[:, 1:2], aff_sb[:, 2:3])
                    return sb

                gsb = iln_affine(gch, lng[:, 0:2], affg_sb, "ga")
                psb = iln_affine(pch, lng[:, 2:4], affp_sb, "pa")

                gsb128 = small.tile([128, 2], f32, tag="gsb128")
                psb128 = small.tile([128, 2], f32, tag="psb128")
                nc.sync.dma_start(out=gsb128[0:32, :], in_=gsb)
                nc.sync.dma_start(out=psb128[0:32, :], in_=psb)
                for o in (32, 64, 96):
                    nc.sync.dma_start(out=gsb128[o : o + 32, :], in_=gsb128[0:32, :])
                    nc.sync.dma_start(out=psb128[o : o + 32, :], in_=psb128[0:32, :])

                # ---- finalize outputs ----
                sgate_sb = tailp.tile([128, 4096], f32, tag="sgate")
                nc.sync.dma_start(out=sgate_sb, in_=sgate_d[:, :])

                upy_sb = tailp.tile([128, 4096], f32, tag="upy")
                nc.scalar.activation(out=upy_sb, in_=greenraw, func=AF.Silu,
                                     bias=gsb128[:, 1:2], scale=gsb128[:, 0:1])
                nc.sync.dma_start(out=upyout_d[:, :], in_=upy_sb)

                zpre = tailp.tile([128, 4096], f32, tag="zpre")
                nc.scalar.activation(out=zpre, in_=purpleraw, func=AF.Sigmoid,
                                     bias=psb128[:, 1:2], scale=psb128[:, 0:1])
                nc.vector.tensor_mul(zpre, zpre, sgate_sb)
                nc.sync.dma_start(out=zout_d[:, :], in_=zpre)

        if loop_n is None:
            emit_body()
        else:
            with tc.For_i(0, loop_n, 1):
                emit_body()

    nc.compile()
    return nc


_NC_CACHE = None
RUN_KWARGS = {}      # test harness may set e.g. {"trace": True}
LAST_RESULTS = None  # BassKernelResults of the most recent run


def kernel(**inputs) -> np.ndarray:
    global _NC_CACHE, LAST_RESULTS
    from concourse.bass_utils import run_bass_kernel_spmd

    if _NC_CACHE is None:
        _NC_CACHE = build_bass()
    nc = _NC_CACHE

    in_maps = []
    for core in _CORES:
        ci = prepare_core_inputs(inputs, core)
        in_maps.append(ci)

    res = run_bass_kernel_spmd(nc, in_maps, _CORES, **RUN_KWARGS)
    LAST_RESULTS = res
    zs = [res.results[c]["zout"] for c in _CORES]
    upys = [res.results[c]["upyout"] for c in _CORES]
    return assemble_output(zs, upys)


if __name__ == "__main__":
    nc = build_bass()
    print("bass build OK")



# revision 9
# speedup vs baseline: 1.3396x; 1.3396x over previous
"""Trainium2 Bass kernel for nn_MultiHeadCrossAttention_47519518163418.

Sharding: 8 cores = (batch b in {0,1}) x (head h in {0..3}); core c: b=c//4, h=c%4.

v2 design (all-bf16 matmuls):
- All PE operands bf16 (enables fast-weight-load; LDWEIGHTS hidden behind MMs).
- QK^T (contraction K=32) uses 4-way PE row tiling (tile_position=(32w,0)):
  4 key-blocks computed concurrently, scores psum [128, 2048] per group.
- exp as one ACTIVATE per [128, 2048] psum group -> bf16 aexp (amortizes the
  ~352-cycle per-instruction overhead).
- PV uses 2-way column tiling (tile_position=(0,0)/(0,64)), stationary
  vT with an appended ones-column (M=33) so softmax row sums come for free;
  two partial output chains summed by one small DVE add per 512-col block.
- green (upsample+conv+ILN+silu) matmuls woven into the attention loop's PE
  slack; purple started mid-loop as soon as its gathered mha rows land.
- mha AllGather in bf16, split in 3 chunks overlapped with attention tail.
- rsqrt computed on DVE via the int32 bit trick + 2 Newton steps (no Sqrt
  activation-table loads).
- host precomputes positional encodings + paddings, casts to bf16, and
  reassembles the sharded bf16 outputs.
"""

import sys

if "/opt/trn_rl_repo" not in sys.path:
    sys.path.insert(0, "/opt/trn_rl_repo")

import numpy as np

try:
    from ml_dtypes import bfloat16 as np_bf16
except ImportError:  # jax ships ml_dtypes
    import jax.numpy as _jnp

    np_bf16 = _jnp.bfloat16

NUM_HEADS = 4
EPS = 1e-5
D_HEAD = 32
SCALE = float(D_HEAD) ** -0.5
N_PX = 16384.0          # pixels per channel of the upsampled image
N_TOT = 128 * 16384.0   # elements per batch for layer stats

_CORES = list(range(8))
_REPLICA_GROUPS = [[0, 1, 2, 3], [4, 5, 6, 7]]

ROW_TILING = True   # 4-way row tiling for QK^T
COL_TILING = True   # 2-way col tiling for PV


# ----------------------------------------------------------------------------
# Host-side helpers
# ----------------------------------------------------------------------------

def pos_encoding_pe(c, L, dtype=np.float32):
    half = c // 2
    pos = np.arange(L, dtype=dtype)
    depths = np.arange(half, dtype=dtype) / half
    rates = 1.0 / (10000.0 ** depths)
    ang = pos[:, None] * rates[None, :]
    pe = np.concatenate([np.sin(ang), np.cos(ang)], axis=-1)  # [L, c]
    return pe.T.astype(dtype)  # [c, L]


def reflect_pad(x):
    return np.pad(x, ((0, 0), (1, 1), (1, 1)), mode="reflect")


def edge_pad(x):
    return np.pad(x, ((0, 0), (1, 1), (1, 1)), mode="edge")


_KSET = {(0, 0): [0], (0, 1): [1, 2], (1, 0): [0, 1], (1, 1): [2]}


def collapse_w2(w):
    """w [co, ci, 3, 3] -> W2 [4 (p=2*pr+pc), 2 (dy), 2 (dx), ci, co]."""
    co, ci = w.shape[0], w.shape[1]
    W2 = np.zeros((4, 2, 2, ci, co), dtype=w.dtype)
    for pr in range(2):
        for pc in range(2):
            p = 2 * pr + pc
            for dy in range(2):
                for dx in range(2):
                    acc = np.zeros((co, ci), dtype=np.float64)
                    for ky in _KSET[(pr, dy)]:
                        for kx in _KSET[(pc, dx)]:
                            acc = acc + w[:, :, ky, kx].astype(np.float64)
                    W2[p, dy, dx] = acc.T.astype(w.dtype)
    return W2


def arrange_to_strips(x2d):
    """x [32, 128, 128] -> arranged [128, 4096] phase-major: partition
    32*(2*pr+pc)+c, free r*64+cc for upsampled pixel (2r+pr, 2cc+pc)."""
    t = x2d.reshape(32, 64, 2, 64, 2)          # c, r, pr, cc, pc
    t = t.transpose(2, 4, 0, 1, 3)              # pr, pc, c, r, cc
    return np.ascontiguousarray(t.reshape(128, 4096))


def unarrange_from_strips(arr):
    t = arr.reshape(2, 2, 32, 64, 64)           # pr, pc, c, r, cc
    t = t.transpose(2, 3, 0, 4, 1)              # c, r, pr, cc, pc
    return np.ascontiguousarray(t.reshape(32, 128, 128))


_PE_Y = None
_PE_S = None


_BATCH_CACHE = {}


def _batch_shared(inputs, b):
    """Padded/PE-added tensors shared by the 4 cores of a batch (bf16)."""
    key = (id(inputs), b)
    if key in _BATCH_CACHE:
        return _BATCH_CACHE[key]
    y = np.asarray(inputs["y"], dtype=np.float32)[b]
    s = np.asarray(inputs["s"], dtype=np.float32)[b]
    ypepad = np.ascontiguousarray(
        reflect_pad((y + _PE_Y).astype(np.float32)).reshape(2, 128, 66, 66)
    ).astype(np_bf16)
    yreppad = np.ascontiguousarray(
        edge_pad(y).reshape(2, 128, 66, 66)).astype(np_bf16)
    spepad = np.ascontiguousarray(
        reflect_pad((s + _PE_S).astype(np.float32))).astype(np_bf16)
    _BATCH_CACHE.clear()
    _BATCH_CACHE[key] = (ypepad, yreppad, spepad)
    return _BATCH_CACHE[key]


def prepare_core_inputs(inputs, core):
    global _PE_Y, _PE_S
    if _PE_Y is None:
        _PE_Y = pos_encoding_pe(256, 64 * 64).reshape(256, 64, 64)
        _PE_S = pos_encoding_pe(128, 128 * 128).reshape(128, 128, 128)
    b, h = core // 4, core % 4
    ch = slice(32 * h, 32 * h + 32)
    s = np.asarray(inputs["s"], dtype=np.float32)[b]

    ypepad, yreppad, spepad = _batch_shared(inputs, b)
    sgate = arrange_to_strips(np.ascontiguousarray(s[ch])).astype(np_bf16)

    w_blue_y = np.asarray(inputs["w_blue_y"], dtype=np.float32)[ch]
    w_blue_s = np.asarray(inputs["w_blue_s"], dtype=np.float32)[ch]
    w_green = np.asarray(inputs["w_green"], dtype=np.float32)[ch]
    w_purple = np.asarray(inputs["w_purple"], dtype=np.float32)[ch]

    wq = np.zeros((18, 128, 128), dtype=np.float32)
    for t in range(9):
        ky, kx = t // 3, t % 3
        for kt in range(2):
            blk = w_blue_y[:, 128 * kt : 128 * kt + 128, ky, kx].T
            wq[t * 2 + kt] = np.tile(blk, (1, 4))
    wv = np.zeros((9, 128, 32), dtype=np.float32)
    for t in range(9):
        ky, kx = t // 3, t % 3
        wv[t] = w_blue_s[:, :, ky, kx].T

    def make_w9(w):
        # W9[ey, ex][ci, 32*p+c] = W2[p, ey-pr, ex-pc][ci, c] (0 if invalid):
        # all four phases computed from one 9-tap pass over the edge-padded
        # original-resolution image, phase-major on output partitions.
        W2 = collapse_w2(w)                      # [4, 2, 2, ci, co32]
        ci = W2.shape[3]
        W9 = np.zeros((3, 3, ci, 128), dtype=np.float32)
        for p in range(4):
            pr, pc = p // 2, p % 2
            for dy in range(2):
                for dx in range(2):
                    W9[pr + dy, pc + dx, :, 32 * p : 32 * p + 32] = W2[p, dy, dx]
        return W9

    W9g = make_w9(w_green)                       # [3, 3, 256, 128]
    wg = W9g.reshape(3, 3, 2, 128, 128).reshape(18, 128, 128).copy()
    W9p = make_w9(w_purple)                      # [3, 3, 128, 128]
    wp = W9p.reshape(9, 128, 128).copy()

    affg = np.stack(
        [np.asarray(inputs["rho_g"], dtype=np.float32).reshape(128)[ch],
         np.asarray(inputs["gamma_g"], dtype=np.float32).reshape(128)[ch],
         np.asarray(inputs["beta_g"], dtype=np.float32).reshape(128)[ch]],
        axis=1)
    affp = np.stack(
        [np.asarray(inputs["rho_p"], dtype=np.float32).reshape(128)[ch],
         np.asarray(inputs["gamma_p"], dtype=np.float32).reshape(128)[ch],
         np.asarray(inputs["beta_p"], dtype=np.float32).reshape(128)[ch]],
        axis=1)

    sel = np.zeros((128, 32), dtype=np.float32)
    for p in range(128):
        sel[p, p % 32] = 1.0

    return {
        "sel": sel,
        "ypepad": ypepad,
        "yreppad": yreppad,
        "spepad": spepad,
        "sgate": sgate,
        "wq": wq.astype(np_bf16),
        "wv": wv.astype(np_bf16),
        "wg": wg.astype(np_bf16),
        "wp": wp.astype(np_bf16),
        "affg": np.ascontiguousarray(affg),
        "affp": np.ascontiguousarray(affp),
    }


def assemble_output(per_core_z, per_core_upy):
    out = np.zeros((2, 256, 128, 128), dtype=np.float32)
    for core in range(8):
        b, h = core // 4, core % 4
        z = np.asarray(per_core_z[core]).astype(np.float32)
        u = np.asarray(per_core_upy[core]).astype(np.float32)
        out[b, 32 * h : 32 * h + 32] = unarrange_from_strips(z)
        out[b, 128 + 32 * h : 128 + 32 * h + 32] = unarrange_from_strips(u)
    return out


# ----------------------------------------------------------------------------
# Bass kernel
# ----------------------------------------------------------------------------

def build_bass(no_cc=False):
    import concourse.bass as bass
    import concourse.tile as tile
    from concourse import bacc, mybir

    f32 = mybir.dt.float32
    bf16 = mybir.dt.bfloat16
    i32 = mybir.dt.int32
    AF = mybir.ActivationFunctionType
    ALU = mybir.AluOpType

    nc = bacc.Bacc(num_devices=8)

    # ---- I/O ----
    ypepad_d = nc.declare_dram_parameter("ypepad", [2, 128, 66, 66], bf16, isOutput=False)
    yreppad_d = nc.declare_dram_parameter("yreppad", [2, 128, 66, 66], bf16, isOutput=False)
    spepad_d = nc.declare_dram_parameter("spepad", [128, 130, 130], bf16, isOutput=False)
    sgate_d = nc.declare_dram_parameter("sgate", [128, 4096], bf16, isOutput=False)
    wq_d = nc.declare_dram_parameter("wq", [18, 128, 128], bf16, isOutput=False)
    wv_d = nc.declare_dram_parameter("wv", [9, 128, 32], bf16, isOutput=False)
    wg_d = nc.declare_dram_parameter("wg", [18, 128, 128], bf16, isOutput=False)
    wp_d = nc.declare_dram_parameter("wp", [9, 128, 128], bf16, isOutput=False)
    affg_d = nc.declare_dram_parameter("affg", [32, 3], f32, isOutput=False)
    affp_d = nc.declare_dram_parameter("affp", [32, 3], f32, isOutput=False)
    sel_d = nc.declare_dram_parameter("sel", [128, 32], f32, isOutput=False)
    zout_d = nc.declare_dram_parameter("zout", [128, 4096], bf16, isOutput=True)
    upyout_d = nc.declare_dram_parameter("upyout", [128, 4096], bf16, isOutput=True)

    # ---- internal DRAM (collective bounce buffers etc.) ----
    # mha AllGather in 3 chunks: cols [0:2048], [2048:3072], [3072:4096]
    CH_COLS = [(0, 2048), (2048, 3072), (3072, 4096)]
    cc_in = [nc.dram_tensor(f"ccg{i}_in", [32, c1 - c0], bf16)
             for i, (c0, c1) in enumerate(CH_COLS)]
    cc_out = [nc.dram_tensor(f"ccg{i}_out", [4, 32, c1 - c0], bf16)
              for i, (c0, c1) in enumerate(CH_COLS)]
    cc2_in = nc.dram_tensor("cc2_in", [1, 4], f32)
    cc2_out = nc.dram_tensor("cc2_out", [1, 4], f32)
    rsumb = nc.dram_tensor("rsumb", [4096], f32)
    rsumb2 = nc.dram_tensor("rsumb2", [4096], f32)

    import contextlib

    with tile.TileContext(nc) as tc, contextlib.ExitStack() as ctx:
        pers = ctx.enter_context(tc.tile_pool(name="pers", bufs=1))
        small = ctx.enter_context(tc.tile_pool(name="small", bufs=1))
        attn = ctx.enter_context(tc.tile_pool(name="attn", bufs=1))

        # ---------------- weights + constants ----------------
        wq_sb = pers.tile([128, 18, 128], bf16, tag="wq")
        nc.sync.dma_start(out=wq_sb, in_=wq_d[:, :, :].rearrange("t p m -> p t m"))
        wv_sb = pers.tile([128, 9, 32], bf16, tag="wv")
        nc.sync.dma_start(out=wv_sb, in_=wv_d[:, :, :].rearrange("t p m -> p t m"))
        wg_sb = pers.tile([128, 18, 128], bf16, tag="wg")
        nc.sync.dma_start(out=wg_sb, in_=wg_d[:, :, :].rearrange("t p m -> p t m"))
        wp_sb = pers.tile([128, 9, 128], bf16, tag="wp")
        nc.sync.dma_start(out=wp_sb, in_=wp_d[:, :, :].rearrange("t p m -> p t m"))
        affg_sb = small.tile([32, 3], f32, tag="affg")
        nc.sync.dma_start(out=affg_sb, in_=affg_d[:, :])
        affp_sb = small.tile([32, 3], f32, tag="affp")
        nc.sync.dma_start(out=affp_sb, in_=affp_d[:, :])

        # Kick the Silu activation-table load at t=0 (hidden under DMA).
        dummy = small.tile([1, 1], f32, tag="dummy")
        nc.vector.memset(dummy, 0.0)
        nc.scalar.activation(out=dummy, in_=dummy, func=AF.Silu)

        def rsqrt_dve(x, p, tag, eps=EPS):
            """[p, 1] fp32 -> 1/sqrt(x+eps) entirely on DVE: int32 bit-trick
            seed + 2 Newton steps (no Sqrt activation table needed)."""
            xe = small.tile([p, 1], f32, tag=tag + "xe", name=tag + "xe")
            nc.vector.tensor_scalar_add(xe, x, eps)
            yi = small.tile([p, 1], i32, tag=tag + "yi", name=tag + "yi")
            # yi = ~(bitcast(xe) >> 1)
            nc.vector.tensor_scalar(out=yi, in0=xe.bitcast(i32), scalar1=1,
                                    scalar2=0xFFFFFFFF,
                                    op0=ALU.logical_shift_right,
                                    op1=ALU.bitwise_xor)
            # yi += magic + 1  (== 0x5f3759df - (x>>1) mod 2^32)
            nc.vector.tensor_scalar_add(yi, yi, 0x5F3759E0)
            y = small.tile([p, 1], f32, tag=tag + "y", name=tag + "y")
            nc.vector.tensor_copy(y, yi.bitcast(f32))
            t = small.tile([p, 1], f32, tag=tag + "nt", name=tag + "nt")
            for _ in range(2):
                nc.vector.tensor_mul(t, y, y)
                nc.vector.tensor_mul(t, t, xe)
                nc.vector.tensor_scalar(out=t, in0=t, scalar1=-0.5, scalar2=1.5,
                                        op0=ALU.mult, op1=ALU.add)
                nc.vector.tensor_mul(y, y, t)
            return y

        # ================= phase 1a: v conv (blue_s, stride 2) ==============
        vT_sb = attn.tile([128, 32, 33], bf16, tag="vT")  # [j, jb, c(+ones)]
        nc.vector.memset(vT_sb[:, :, 32:33], 1.0)
        vstats = small.tile([32, 8, 6], f32, tag="vstats")
        with tc.tile_pool(name="vsec", bufs=1) as vsec, \
             tc.tile_pool(name="vtmp", bufs=2) as vtmp_pool, \
             tc.tile_pool(name="cps2", bufs=3, space="PSUM") as cps2:
            spe = vsec.tile([128, 130, 130], bf16, tag="spe")
            for rb in range(5):
                r0, r1 = 26 * rb, 26 * rb + 26
                nc.sync.dma_start(out=spe[:, r0:r1, :], in_=spepad_d[:, r0:r1, :])
            vraw = vsec.tile([32, 4096], f32, tag="vraw")
            for chunk in range(8):
                vps = cps2.tile([128, 512], f32, tag="vps")
                r0 = 8 * chunk
                for t in range(9):
                    ky, kx = t // 3, t % 3
                    nc.tensor.matmul(
                        vps[0:32, :],
                        wv_sb[:, t, :],
                        spe[:, 2 * r0 + ky : 2 * r0 + ky + 16 : 2, kx : kx + 128 : 2],
                        start=(t == 0), stop=(t == 8),
                    )
                nc.vector.tensor_copy(vraw[:, 512 * chunk : 512 * chunk + 512], vps[0:32, :])
                nc.vector.bn_stats(out=vstats[:, chunk, :], in_=vraw[:, 512 * chunk : 512 * chunk + 512])

            vmv = small.tile([32, 2], f32, tag="vmv")
            nc.vector.bn_aggr(out=vmv, in_=vstats)
            vinv = rsqrt_dve(vmv[:, 1:2], 32, "vinv")
            vbias = small.tile([32, 1], f32, tag="vbias")
            nc.vector.tensor_scalar(out=vbias, in0=vmv[:, 0:1], scalar1=vinv, scalar2=-1.0,
                                    op0=ALU.mult, op1=ALU.mult)
            v2d = vtmp_pool.tile([32, 4096], bf16, tag="vtmp", name="v2d")
            nc.scalar.activation(out=v2d, in_=vraw, func=AF.Silu, bias=vbias, scale=vinv)

            # vT with ones column: vT_sb[32w+i, jb, c] = v2d[c, 128*jb+32w+i]
            vt32 = vtmp_pool.tile([32, 4096], bf16, tag="vtmp", name="vt32")
            nc.vector.transpose(out=vt32, in_=v2d)
            vt32_v = vt32.rearrange("p (m c) -> p m c", c=32)   # m = 4*jb + w
            for w in range(4):
                nc.gpsimd.dma_start(
                    out=vT_sb[32 * w : 32 * w + 32, :, 0:32],
                    in_=vt32_v[:, w::4, :])

        # ================= phase 1b: q conv (blue_y, M=128 replicated) ======
        qrep = attn.tile([128, 4096], bf16, tag="qrep")
        qstats = small.tile([128, 8, 6], f32, tag="qstats")
        with tc.tile_pool(name="qsec", bufs=1) as qsec, \
             tc.tile_pool(name="cps1", bufs=3, space="PSUM") as cps1:
            ype = [qsec.tile([128, 66, 66], bf16, tag=f"ype{kt}", name=f"ype{kt}") for kt in range(2)]
            for kt in range(2):
                for rb in range(3):
                    r0, r1 = 22 * rb, 22 * rb + 22
                    nc.sync.dma_start(out=ype[kt][:, r0:r1, :], in_=ypepad_d[kt][:, r0:r1, :])
            qraw = qsec.tile([128, 4096], f32, tag="qraw")
            for chunk in range(8):
                qps = cps1.tile([128, 512], f32, tag="qps")
                r0 = 8 * chunk
                idx = 0
                for t in range(9):
                    ky, kx = t // 3, t % 3
                    for kt in range(2):
                        nc.tensor.matmul(
                            qps[:, :],
                            wq_sb[:, t * 2 + kt, :],
                            ype[kt][:, r0 + ky : r0 + ky + 8, kx : kx + 64],
                            start=(idx == 0), stop=(idx == 17),
                        )
                        idx += 1
                nc.vector.tensor_copy(qraw[:, 512 * chunk : 512 * chunk + 512], qps[:, :])
                nc.vector.bn_stats(out=qstats[:, chunk, :], in_=qraw[:, 512 * chunk : 512 * chunk + 512])

            qmv = small.tile([128, 2], f32, tag="qmv")
            nc.vector.bn_aggr(out=qmv, in_=qstats)
            qinv = rsqrt_dve(qmv[:, 1:2], 128, "qinv")
            qbias = small.tile([128, 1], f32, tag="qbias")
            nc.vector.tensor_scalar(out=qbias, in0=qmv[:, 0:1], scalar1=qinv, scalar2=-1.0,
                                    op0=ALU.mult, op1=ALU.mult)
            nc.scalar.activation(out=qrep, in_=qraw, func=AF.Silu, bias=qbias, scale=qinv)

        # ================= phase 2: attention + green conv ==================
        greenraw = ctx.enter_context(tc.tile_pool(name="gpool", bufs=1)).tile(
            [128, 4096], f32, tag="greenraw", name="greenraw")
        gstats = small.tile([128, 8, 6], f32, tag="gstats")
        purpleraw = ctx.enter_context(tc.tile_pool(name="ppool", bufs=1)).tile(
            [128, 4096], f32, tag="purpleraw", name="purpleraw")
        pstats = small.tile([128, 8, 6], f32, tag="pstats")
        mharaw = attn.tile([33, 4096], f32, tag="mharaw")
        mha2db = attn.tile([32, 4096], bf16, tag="mha2db")
        mhapad = attn.tile([128, 66, 66], bf16, tag="mhapad")

        with tc.tile_pool(name="yrep", bufs=1) as yrep_pool, \
             tc.tile_pool(name="aexpp", bufs=2) as aexp_pool, \
             tc.tile_pool(name="qkps", bufs=1, space="PSUM") as qkps, \
             tc.tile_pool(name="pvps", bufs=2, space="PSUM") as pvps, \
             tc.tile_pool(name="gps", bufs=2, space="PSUM") as gps_pool:
            yrep = [yrep_pool.tile([128, 66, 66], bf16, tag=f"yrep{kt}", name=f"yrep{kt}") for kt in range(2)]
            for kt in range(2):
                for rb in range(3):
                    r0, r1 = 22 * rb, 22 * rb + 22
                    nc.sync.dma_start(out=yrep[kt][:, r0:r1, :], in_=yreppad_d[kt][:, r0:r1, :])

            green_tiles = {}

            def green_piece(piece):
                # piece = (chunk, sub) with sub in 0..5 -> 3 MMs each
                chunk, sub = piece // 6, piece % 6
                r0 = 8 * chunk
                if sub == 0:
                    green_tiles[chunk] = gps_pool.tile(
                        [128, 512], f32, tag="convps", name=f"g{chunk}")
                gtile = green_tiles[chunk]
                for k in range(3):
                    idx = sub * 3 + k
                    tap, kt = idx // 2, idx % 2
                    ey, ex = tap // 3, tap % 3
                    nc.tensor.matmul(
                        gtile[:, :],
                        wg_sb[:, tap * 2 + kt, :],
                        yrep[kt][:, r0 + ey : r0 + ey + 8, ex : ex + 64],
                        start=(idx == 0), stop=(idx == 17),
                    )
                if sub == 5:
                    col = 512 * chunk
                    nc.vector.tensor_copy(greenraw[:, col : col + 512], gtile[:, :])
                    nc.vector.bn_stats(out=gstats[:, chunk, :], in_=greenraw[:, col : col + 512])
                    del green_tiles[chunk]

            purple_tiles = {}

            def purple_piece(piece):
                # piece = (chunk, sub) with sub in 0..2 -> 3 MMs each
                chunk, sub = piece // 3, piece % 3
                r0 = 8 * chunk
                if sub == 0:
                    purple_tiles[chunk] = gps_pool.tile(
                        [128, 512], f32, tag="convps", name=f"p{chunk}")
                ptile = purple_tiles[chunk]
                for k in range(3):
                    tap = sub * 3 + k
                    ey, ex = tap // 3, tap % 3
                    nc.tensor.matmul(
                        ptile[:, :],
                        wp_sb[:, tap, :],
                        mhapad[:, r0 + ey : r0 + ey + 8, ex : ex + 64],
                        start=(tap == 0), stop=(tap == 8),
                    )
                if sub == 2:
                    col = 512 * chunk
                    nc.vector.tensor_copy(purpleraw[:, col : col + 512], ptile[:, :])
                    nc.vector.bn_stats(out=pstats[:, chunk, :], in_=purpleraw[:, col : col + 512])
                    del purple_tiles[chunk]

            def emit_qk(gi):
                I, g = gi // 8, gi % 8
                qkt = qkps.tile([128, 2048], f32, tag="qk", name=f"qk{gi}")
                for w in range(4):
                    jb = 4 * g + w
                    tp = (32 * w, 0) if ROW_TILING else None
                    nc.tensor.matmul(
                        qkt[:, 512 * w : 512 * w + 512],
                        qrep[32 * w : 32 * w + 32, 128 * jb : 128 * jb + 128],
                        qrep[32 * w : 32 * w + 32, 512 * I : 512 * I + 512],
                        start=True, stop=True,
                        tile_position=tp,
                        skip_group_check=True,
                    )
                return qkt

            def emit_exp(gi, qkt):
                aexp = aexp_pool.tile([128, 2048], bf16, tag="aexp", name=f"ae{gi}")
                nc.scalar.activation(out=aexp, in_=qkt, func=AF.Exp, scale=SCALE)
                return aexp

            pv_tiles = {}

            def emit_pv(gi, aexp):
                I, g = gi // 8, gi % 8
                if g == 0:
                    pv_tiles[I] = pvps.tile([128, 512], f32, tag="pvt", name=f"pvt{I}")
                pvt = pv_tiles[I]
                for t in range(2):
                    for k in range(2):       # k=0 -> chain A (base 0), k=1 -> chain B (base 64)
                        w = 2 * t + k
                        jb = 4 * g + w
                        base = 64 * k if COL_TILING else 0
                        tp = (0, base) if COL_TILING else None
                        nc.tensor.matmul(
                            pvt[base : base + 33, :],
                            vT_sb[:, jb, :],
                            aexp[:, 512 * w : 512 * w + 512],
                            start=(g == 0 and t == 0), stop=(g == 7 and t == 1),
                            tile_position=tp,
                            skip_group_check=True,
                        )

            def finish_I(I):
                # combine the two PV partial chains; normalize; stash bf16 mha
                pvt = pv_tiles.pop(I)
                c0 = 512 * I
                nc.vector.tensor_copy(mharaw[:, c0 : c0 + 512], pvt[0:33, :])
                if COL_TILING:
                    # chain B sits at partitions 64:97; engines cannot shift
                    # partitions, so copy it out partition-aligned and fold it
                    # in with a gpsimd accumulate-DMA.
                    mhtmp = small.tile([128, 512], f32, tag="mhtmp", name=f"mhtmp{I}")
                    nc.vector.tensor_copy(mhtmp[64:97, :], pvt[64:97, :])
                    nc.gpsimd.dma_start(out=mharaw[:, c0 : c0 + 512],
                                        in_=mhtmp[64:97, :], accum_op=ALU.add)
                # rowsum -> DRAM -> [128, 4] recip -> DRAM -> [32, 512] bcast
                nc.sync.dma_start(
                    out=bass.AP(tensor=rsumb, offset=c0, ap=[[1, 512]]),
                    in_=mharaw[32:33, c0 : c0 + 512])
                rsq = small.tile([128, 4], f32, tag="rsq", name=f"rsq{I}")
                nc.sync.dma_start(
                    out=rsq, in_=bass.AP(tensor=rsumb, offset=c0, ap=[[4, 128], [1, 4]]))
                nc.vector.reciprocal(out=rsq, in_=rsq)
                nc.sync.dma_start(
                    out=bass.AP(tensor=rsumb2, offset=c0, ap=[[4, 128], [1, 4]]), in_=rsq)
                rs32 = small.tile([32, 512], f32, tag="rs32", name=f"rs32_{I}")
                nc.sync.dma_start(
                    out=rs32, in_=bass.AP(tensor=rsumb2, offset=c0, ap=[[0, 32], [1, 512]]))
                nc.vector.tensor_mul(mha2db[:, c0 : c0 + 512], mharaw[0:32, c0 : c0 + 512], rs32)

            def send_chunk(ci):
                c0, c1 = CH_COLS[ci]
                nc.sync.dma_start(out=cc_in[ci][:, :], in_=mha2db[:, c0:c1])
                if no_cc:
                    for gidx in range(4):
                        nc.sync.dma_start(out=cc_out[ci][gidx], in_=cc_in[ci][:, :])
                else:
                    nc.gpsimd.collective_compute(
                        "AllGather", mybir.AluOpType.bypass,
                        replica_groups=_REPLICA_GROUPS,
                        ins=[cc_in[ci][:, :]],
                        outs=[cc_out[ci][:, :, :]],
                    )
                # land into mhapad [128, 66, 66] (+1 row/col offsets), with pads
                ccv = cc_out[ci].rearrange("h c (r x) -> (h c) r x", x=64)
                nrows = (c1 - c0) // 512 * 8
                row0 = c0 // 512 * 8
                nc.sync.dma_start(out=mhapad[:, 1 + row0 : 1 + row0 + nrows, 1:65], in_=ccv)
                if ci == 0:
                    nc.sync.dma_start(out=mhapad[:, 0:1, 1:65], in_=ccv[:, 0:1, :])
                if ci == 2:
                    nc.sync.dma_start(out=mhapad[:, 65:66, 1:65], in_=ccv[:, nrows - 1 : nrows, :])
                # edge columns for the landed rows (incl. top/bottom pad rows)
                r0 = row0 if ci > 0 else 0
                r1 = 1 + row0 + nrows if ci < 2 else 66
                nc.sync.dma_start(out=mhapad[:, r0:r1, 0:1], in_=mhapad[:, r0:r1, 1:2])
                nc.sync.dma_start(out=mhapad[:, r0:r1, 65:66], in_=mhapad[:, r0:r1, 64:65])

            # -------- main attention loop --------
            # pipeline: iter gi emits QK(gi), exp(gi), PV(gi-1), filler MMs
            prev = None  # (gi, aexp)
            gfill = 0    # next green piece (48 total)
            pfill = 0    # next purple piece (24 total; chunks gated by DMA deps)
            for gi in range(64):
                qkt = emit_qk(gi)
                aexp = emit_exp(gi, qkt)
                if prev is not None:
                    emit_pv(prev[0], prev[1])
                    if prev[0] % 8 == 7:
                        I = prev[0] // 8
                        finish_I(I)
                        if I == 3:
                            send_chunk(0)
                        elif I == 5:
                            send_chunk(1)
                prev = (gi, aexp)
                # PE filler: one green piece per iter (48 total); purple
                # pieces only well after their gather chunk was sent, so an
                # in-order PE never stalls on an in-flight collective.
                if gfill < 48:
                    green_piece(gfill)
                    gfill += 1
                plimit = 0 if gi < 50 else (9 if gi < 59 else 14)
                if pfill < plimit:
                    purple_piece(pfill)
                    pfill += 1
            emit_pv(prev[0], prev[1])
            finish_I(7)
            send_chunk(2)

            # remaining purple pieces (chunks 5..7 wait on gather chunk C)
            while pfill < 24:
                purple_piece(pfill)
                pfill += 1

        # ================= phase 3: stats combine + finalize ================
        with tc.tile_pool(name="tailp", bufs=1) as tailp, \
             tc.tile_pool(name="tailps", bufs=2, space="PSUM") as tailps:
            gmv = small.tile([128, 2], f32, tag="gmv")
            nc.vector.bn_aggr(out=gmv, in_=gstats)
            pmv = small.tile([128, 2], f32, tag="pmv")
            nc.vector.bn_aggr(out=pmv, in_=pstats)

            sel_sb = pers.tile([128, 32], f32, tag="sel")
            nc.sync.dma_start(out=sel_sb, in_=sel_d[:, :])
            ones_sb = pers.tile([128, 1], f32, tag="ones")
            nc.vector.memset(ones_sb, 1.0)

            # ---- stats -> sums, phase-group combine (PE), layer sums ----
            def part_sums(mv, tag):
                s2 = small.tile([128, 2], f32, tag=tag, name=tag)
                nc.vector.tensor_scalar_mul(s2[:, 0:1], mv[:, 0:1], 4096.0)
                t = small.tile([128, 1], f32, tag=tag + "t", name=tag + "t")
                nc.vector.tensor_mul(t, mv[:, 0:1], mv[:, 0:1])
                nc.vector.tensor_add(t, t, mv[:, 1:2])
                nc.vector.tensor_scalar_mul(s2[:, 1:2], t, 4096.0)
                return s2

            gsums2 = part_sums(gmv, "gsums2")
            psums2 = part_sums(pmv, "psums2")

            chps = tailps.tile([128, 512], f32, tag="tps", name="chps")
            nc.tensor.matmul(chps[0:32, 0:2], sel_sb, gsums2, start=True, stop=True)
            gch = small.tile([32, 2], f32, tag="gch")
            nc.vector.tensor_copy(gch, chps[0:32, 0:2])
            chps2 = tailps.tile([128, 512], f32, tag="tps", name="chps2")
            nc.tensor.matmul(chps2[0:32, 0:2], sel_sb, psums2, start=True, stop=True)
            pch = small.tile([32, 2], f32, tag="pch")
            nc.vector.tensor_copy(pch, chps2[0:32, 0:2])

            lps = tailps.tile([128, 512], f32, tag="tps", name="lps")
            nc.tensor.matmul(lps[0:1, 0:2], ones_sb, gsums2, start=True, stop=True)
            nc.tensor.matmul(lps[0:1, 2:4], ones_sb, psums2, start=True, stop=True)
            lsb = small.tile([1, 4], f32, tag="lsb")
            nc.vector.tensor_copy(lsb, lps[0:1, 0:4])
            nc.sync.dma_start(out=cc2_in[:, :], in_=lsb)
            if no_cc:
                nc.sync.dma_start(out=cc2_out[:, :], in_=cc2_in[:, :])
            else:
                nc.gpsimd.collective_compute(
                    "AllReduce", mybir.AluOpType.add,
                    replica_groups=_REPLICA_GROUPS,
                    ins=[cc2_in[:, :]],
                    outs=[cc2_out[:, :]],
                )
            lng = small.tile([32, 4], f32, tag="lng")
            nc.sync.dma_start(out=lng, in_=bass.AP(tensor=cc2_out, offset=0, ap=[[0, 32], [1, 4]]))

            # ---- ILN affines ----
            def iln_affine(ch_sums, S_col, aff_sb, tag):
                n, n1 = N_PX, N_PX - 1.0
                nt, nt1 = N_TOT, N_TOT - 1.0
                in_m = small.tile([32, 1], f32, tag=tag + "im", name=tag + "im")
                nc.vector.tensor_scalar_mul(in_m, ch_sums[:, 0:1], 1.0 / n)
                t1 = small.tile([32, 1], f32, tag=tag + "t1", name=tag + "t1")
                nc.vector.tensor_mul(t1, ch_sums[:, 0:1], ch_sums[:, 0:1])
                nc.vector.tensor_scalar_mul(t1, t1, 1.0 / n)
                nc.vector.tensor_sub(t1, ch_sums[:, 1:2], t1)
                in_v = small.tile([32, 1], f32, tag=tag + "iv", name=tag + "iv")
                nc.vector.tensor_scalar_mul(in_v, t1, 1.0 / n1)
                inv_in = rsqrt_dve(in_v, 32, tag + "ii")

                ln_m = small.tile([32, 1], f32, tag=tag + "lm", name=tag + "lm")
                nc.vector.tensor_scalar_mul(ln_m, S_col[:, 0:1], 1.0 / nt)
                l1 = small.tile([32, 1], f32, tag=tag + "l1", name=tag + "l1")
                nc.vector.tensor_mul(l1, S_col[:, 0:1], S_col[:, 0:1])
                nc.vector.tensor_scalar_mul(l1, l1, 1.0 / nt)
                nc.vector.tensor_sub(l1, S_col[:, 1:2], l1)
                ln_v = small.tile([32, 1], f32, tag=tag + "lv", name=tag + "lv")
                nc.vector.tensor_scalar_mul(ln_v, l1, 1.0 / nt1)
                inv_ln = rsqrt_dve(ln_v, 32, tag + "il")

                rho = aff_sb[:, 0:1]
                t3 = small.tile([32, 1], f32, tag=tag + "t3", name=tag + "t3")
                nc.vector.tensor_mul(t3, rho, inv_in)
                t6 = small.tile([32, 1], f32, tag=tag + "t6", name=tag + "t6")
                nc.vector.tensor_mul(t6, rho, inv_ln)
                nc.vector.tensor_sub(t6, inv_ln, t6)
                A = small.tile([32, 1], f32, tag=tag + "A", name=tag + "A")
                nc.vector.tensor_add(A, t3, t6)
                u1 = small.tile([32, 1], f32, tag=tag + "u1", name=tag + "u1")
                nc.vector.tensor_mul(u1, in_m, t3)
                u2 = small.tile([32, 1], f32, tag=tag + "u2", name=tag + "u2")
                nc.vector.tensor_mul(u2, ln_m, t6)
                nc.vector.tensor_add(u1, u1, u2)
                B = small.tile([32, 1], f32, tag=tag + "B", name=tag + "B")
                nc.vector.tensor_scalar_mul(B, u1, -1.0)
                sb = small.tile([32, 2], f32, tag=tag + "sb", name=tag + "sb")
                nc.vector.tensor_mul(sb[:, 0:1], A, aff_sb[:, 1:2])
                nc.vector.tensor_mul(sb[:, 1:2], B, aff_sb[:, 1:2])
                nc.vector.tensor_add(sb[:, 1:2], sb[:, 1:2], aff_sb[:, 2:3])
                return sb

            gsb = iln_affine(gch, lng[:, 0:2], affg_sb, "ga")
            psb = iln_affine(pch, lng[:, 2:4], affp_sb, "pa")

            gsb128 = small.tile([128, 2], f32, tag="gsb128")
            psb128 = small.tile([128, 2], f32, tag="psb128")
            nc.sync.dma_start(out=gsb128[0:32, :], in_=gsb)
            nc.sync.dma_start(out=psb128[0:32, :], in_=psb)
            for o in (32, 64, 96):
                nc.sync.dma_start(out=gsb128[o : o + 32, :], in_=gsb128[0:32, :])
                nc.sync.dma_start(out=psb128[o : o + 32, :], in_=psb128[0:32, :])

            # ---- finalize outputs (purple/zout first: longer dep chain) ----
            sgate_sb = tailp.tile([128, 4096], bf16, tag="sgate")
            nc.sync.dma_start(out=sgate_sb, in_=sgate_d[:, :])

            zpre = tailp.tile([128, 4096], bf16, tag="zpre")
            nc.scalar.activation(out=zpre, in_=purpleraw, func=AF.Sigmoid,
                                 bias=psb128[:, 1:2], scale=psb128[:, 0:1])
            nc.vector.tensor_mul(zpre, zpre, sgate_sb)
            nc.sync.dma_start(out=zout_d[:, :], in_=zpre)

            upy_sb = tailp.tile([128, 4096], bf16, tag="upy")
            nc.scalar.activation(out=upy_sb, in_=greenraw, func=AF.Silu,
                                 bias=gsb128[:, 1:2], scale=gsb128[:, 0:1])
            nc.sync.dma_start(out=upyout_d[:, :], in_=upy_sb)

    nc.compile()
    return nc


_NC_CACHE = None
RUN_KWARGS = {}      # test harness may set e.g. {"trace": True}
LAST_RESULTS = None  # BassKernelResults of the most recent run


def kernel(**inputs) -> np.ndarray:
    global _NC_CACHE, LAST_RESULTS
    from concourse.bass_utils import run_bass_kernel_spmd

    if _NC_CACHE is None:
        _NC_CACHE = build_bass()
    nc = _NC_CACHE

    in_maps = []
    for core in _CORES:
        ci = prepare_core_inputs(inputs, core)
        in_maps.append(ci)

    res = run_bass_kernel_spmd(nc, in_maps, _CORES, **RUN_KWARGS)
    LAST_RESULTS = res
    zs = [res.results[c]["zout"] for c in _CORES]
    upys = [res.results[c]["upyout"] for c in _CORES]
    return assemble_output(zs, upys)


if __name__ == "__main__":
    nc = build_bass()
    print("bass build OK")
